# revision 1
# baseline (speedup 1.0000x reference)
"""Multi-head attention (B=2, N=2048, E=1024, H=16) on 8 Trainium2 NeuronCores.

Sharding: data-parallel over batch (2) x tensor-parallel over head-groups (4
groups of 4 heads).  Core c handles batch c//4 and heads 4*(c%4)..4*(c%4)+3.

Host-side shard prep packs ALL per-core inputs (feature-major fp16
activations, transposed fp16 weight shards, fp16 biases) into a single flat
fp16 blob — one input parameter + one output parameter minimizes the
per-parameter dispatch cost of each execution.  The device kernel computes
  qT = Wq_s @ xT + bq_s        (feature-major, [256, 2048], fp16)
  kT = Wk_s @ xT + bk_s
  v  = x @ Wv_s.T + bv_s       (position-major, [2048, 256], bf16)
  eT[kpos, q] per head          (transposed energy, head pairs row-packed
                                 into the PE array, K=64 each)
  s = exp(eT)  (bf16)           (no max-subtraction: |logits| < ~60 << 88)
  o  = s.T @ [v | 32]           (32-column yields 32*rowsum in psum row 64)
  oT normalized by 1/(32*rowsum)   (= softmax / sqrt(E) module quirk)
  out_partial = oT.T @ Wp[:, cols].T   (position-major [2048, 1024], fp16)
Host sums the 4 head-group partials per batch and adds bp.

All matmuls run with 16-bit operands (fp16 where range allows, bf16 for the
attention weights whose unnormalized exp can reach ~e^50); PSUM accumulation
is fp32.  q-chunk projections and the output projection are emitted
*interleaved* with the attention inner loop so the PE fills the gaps while
ScalarE (exp) runs.
"""

import numpy as np

B, N, E, H = 2, 2048, 1024, 16
D = E // H           # 64
NCORES = 8
HG = 4               # head groups
DH = E // HG         # 256 features per head-group
P = 128
NCH = N // 512       # 4 n-chunks of 512
ECH = E // P         # 8 contraction chunks
DCH = DH // P        # 2 feature chunks per shard
KT = N // P          # 16 key tiles
SCALE_COL = float(E ** 0.5)   # 32.0; row 64 of po = 32*rowsum

# flat fp16 blob layout (element offsets).  Each core carries only its
# 512-position slice of x; the 4-core batch group AllGathers the full x
# on-device (D2D bandwidth is cheap; host->device bound bytes are not).
SZ_X = E * N          # 2097152 (full, after gather)
SZ_XS = SZ_X // HG    # 524288 per-core slice
SZ_W = E * DH         # 262144
SZ_WP = DH * E        # 262144
OFF_XQ = 0
OFF_XK = OFF_XQ + SZ_XS
OFF_XV = OFF_XK + SZ_XS
OFF_WQ = OFF_XV + SZ_XS
OFF_WK = OFF_WQ + SZ_W
OFF_WV = OFF_WK + SZ_W
OFF_WP = OFF_WV + SZ_W
OFF_BQ = OFF_WP + SZ_WP
OFF_BK = OFF_BQ + DH
OFF_BV = OFF_BK + DH
SZ_BLOB = OFF_BV + DH

_CACHE = {}


def _build_program():
    import concourse.bacc as bacc
    import concourse.tile as tile
    from concourse import mybir

    F32 = mybir.dt.float32
    F16 = mybir.dt.float16
    BF16 = mybir.dt.bfloat16
    EXP = mybir.ActivationFunctionType.Exp

    nc = bacc.Bacc(None, target_bir_lowering=False, debug=False,
                   enable_partition_id=False, num_devices=NCORES)

    blob = nc.declare_dram_parameter("blob", [SZ_BLOB], F16, isOutput=False)
    out = nc.declare_dram_parameter("out", [N, E], F16, isOutput=True)

    # dram views into the blob
    wqt = blob[OFF_WQ : OFF_WQ + SZ_W].rearrange("(c p m) -> p c m", c=ECH, p=P)
    wkt = blob[OFF_WK : OFF_WK + SZ_W].rearrange("(c p m) -> p c m", c=ECH, p=P)
    wvt = blob[OFF_WV : OFF_WV + SZ_W].rearrange("(c p m) -> p c m", c=ECH, p=P)
    wpt = blob[OFF_WP : OFF_WP + SZ_WP].rearrange("(c p m) -> p c m", c=DCH, p=P)
    bqv = blob[OFF_BQ : OFF_BQ + DH].rearrange("(c p) -> p c", p=P)
    bkv = blob[OFF_BK : OFF_BK + DH].rearrange("(c p) -> p c", p=P)
    bvv = blob[OFF_BV : OFF_BV + DH].rearrange("(a m) -> a m", a=1)

    RG = [[g * HG + r for r in range(HG)] for g in range(NCORES // HG)]

    with tile.TileContext(nc) as tc:
        with (
            nc.allow_low_precision(reason="16-bit activations; tol 2e-2"),
            tc.tile_pool(name="dram", bufs=1, space="DRAM") as dpool,
            tc.tile_pool(name="singles", bufs=1) as singles,
            tc.tile_pool(name="xpool", bufs=3) as xpool,
            tc.tile_pool(name="spool", bufs=4) as spool,
            tc.tile_pool(name="npool", bufs=2) as npool,
            tc.tile_pool(name="opool", bufs=2) as opool,
            tc.tile_pool(name="pproj", bufs=1, space="PSUM") as pproj,
            tc.tile_pool(name="peps", bufs=2, space="PSUM") as peps,
            tc.tile_pool(name="ppo", bufs=2, space="PSUM") as ppo,
            tc.tile_pool(name="pbc", bufs=1, space="PSUM") as pbc,
        ):
            # ---- AllGather the full x from per-core position slices ----
            gath = {}
            for nm, off in (("xk", OFF_XK), ("xv", OFF_XV), ("xq", OFF_XQ)):
                bi = dpool.tile([SZ_XS], F16, name=f"bi_{nm}")
                bg = dpool.tile([SZ_X], F16, name=f"bg_{nm}")
                nc.sync.dma_start(out=bi[:], in_=blob[off : off + SZ_XS])
                nc.gpsimd.collective_compute(
                    "AllGather",
                    mybir.AluOpType.bypass,
                    replica_groups=RG,
                    ins=[bi.opt()],
                    outs=[bg.opt()],
                )
                gath[nm] = bg
            # gathered section r holds positions [r*512, (r+1)*512) feature-major
            def x_chunk_ap(nm, ni):
                return gath[nm][ni * SZ_XS : (ni + 1) * SZ_XS].rearrange(
                    "(c p m) -> p c m", c=ECH, p=P
                )

            # ---- persistent weights / biases ----
            wq_sb = singles.tile([P, ECH, DH], F16)
            wk_sb = singles.tile([P, ECH, DH], F16)
            wv_sb = singles.tile([P, ECH, DH], F16)
            wp_sb = singles.tile([P, DCH, E], F16)
            nc.sync.dma_start(out=wq_sb, in_=wqt)
            nc.sync.dma_start(out=wk_sb, in_=wkt)
            nc.sync.dma_start(out=wv_sb, in_=wvt)
            nc.sync.dma_start(out=wp_sb, in_=wpt)
            bq16 = singles.tile([P, DCH], F16)
            bk16 = singles.tile([P, DCH], F16)
            nc.sync.dma_start(out=bq16, in_=bqv)
            nc.sync.dma_start(out=bk16, in_=bkv)
            bq_sb = singles.tile([P, DCH], F32)
            bk_sb = singles.tile([P, DCH], F32)
            nc.vector.tensor_copy(bq_sb, bq16)
            nc.vector.tensor_copy(bk_sb, bk16)
            bv_sb = singles.tile([1, DH], F16)
            nc.sync.dma_start(out=bv_sb, in_=bvv)
            ones1 = singles.tile([1, P], F16)
            nc.vector.memset(ones1, 1.0)
            ones1_b = singles.tile([1, P], BF16)
            nc.vector.memset(ones1_b, 1.0)

            qT_sb = singles.tile([P, DCH, N], F16)
            kT_sb = singles.tile([P, DCH, N], F16)
            oT_sb = singles.tile([P, DCH, N], F16)
            v_sb = singles.tile([P, KT, HG, D + 1], BF16)
            nc.vector.memset(v_sb[:, :, :, D : D + 1], SCALE_COL)

            # ---- emit helpers ----
            def emit_kv_chunk(ni):
                ns = slice(ni * 512, (ni + 1) * 512)
                xk_c = xpool.tile([P, ECH, 512], F16, tag="x", name=f"xk{ni}")
                nc.sync.dma_start(out=xk_c, in_=x_chunk_ap("xk", ni))
                for dc in range(DCH):
                    ps = pproj.tile([P, 512], F32, tag="proj", name=f"kps{ni}{dc}")
                    for ec in range(ECH):
                        nc.tensor.matmul(
                            ps,
                            wk_sb[:, ec, dc * P : (dc + 1) * P],
                            xk_c[:, ec, :],
                            start=(ec == 0),
                            stop=(ec == ECH - 1),
                        )
                    nc.vector.tensor_scalar_add(
                        kT_sb[:, dc, ns], ps, bk_sb[:, dc : dc + 1]
                    )
                xv_c = xpool.tile([P, ECH, 512], F16, tag="x", name=f"xv{ni}")
                nc.sync.dma_start(out=xv_c, in_=x_chunk_ap("xv", ni))
                for k4 in range(4):
                    kt = ni * 4 + k4
                    vps = pproj.tile([P, DH], F32, tag="proj", name=f"vps{kt}")
                    nc.tensor.matmul(vps, ones1, bv_sb, start=True, stop=False)
                    for ec in range(ECH):
                        nc.tensor.matmul(
                            vps,
                            xv_c[:, ec, k4 * P : (k4 + 1) * P],
                            wv_sb[:, ec, :],
                            start=False,
                            stop=(ec == ECH - 1),
                        )
                    nc.vector.tensor_copy(
                        v_sb[:, kt, :, 0:D],
                        vps.rearrange("p (h d) -> p h d", h=HG),
                    )

            def q_proj_units(ni):
                """Deferred q-projection for chunk ni: DMA + one unit per dc."""
                ns = slice(ni * 512, (ni + 1) * 512)
                state = {}

                def dma_unit():
                    xq_c = xpool.tile([P, ECH, 512], F16, tag="x", name=f"xq{ni}")
                    nc.sync.dma_start(out=xq_c, in_=x_chunk_ap("xq", ni))
                    state["xq"] = xq_c

                def unit(dc):
                    xq_c = state["xq"]
                    ps = pproj.tile([P, 512], F32, tag="proj", name=f"qps{ni}{dc}")
                    for ec in range(ECH):
                        nc.tensor.matmul(
                            ps,
                            wq_sb[:, ec, dc * P : (dc + 1) * P],
                            xq_c[:, ec, :],
                            start=(ec == 0),
                            stop=(ec == ECH - 1),
                        )
                    nc.vector.tensor_scalar_add(
                        qT_sb[:, dc, ns], ps, bq_sb[:, dc : dc + 1]
                    )

                return [dma_unit] + [lambda dc=dc: unit(dc) for dc in range(DCH)]

            def outproj_units(qc):
                """Deferred output projection for q-chunk qc: 4 n-tile units."""

                def unit(nt):
                    n0 = qc * 512 + nt * P
                    osb = opool.tile([P, E], F16, tag="osb", name=f"osb{qc}{nt}")
                    for ecx in range(2):
                        ops = pproj.tile(
                            [P, 512], F32, tag="proj", name=f"ops{qc}{nt}{ecx}"
                        )
                        for dc in range(DCH):
                            nc.tensor.matmul(
                                ops,
                                oT_sb[:, dc, n0 : n0 + P],
                                wp_sb[:, dc, ecx * 512 : (ecx + 1) * 512],
                                start=(dc == 0),
                                stop=(dc == DCH - 1),
                            )
                        nc.vector.tensor_copy(
                            osb[:, ecx * 512 : (ecx + 1) * 512], ops
                        )
                    nc.sync.dma_start(out=out[n0 : n0 + P, :], in_=osb)

                return [lambda nt=nt: unit(nt) for nt in range(4)]

            def attn_groups(qc, pr, po, ktgs, slots=None, si0=0):
                qs = slice(qc * 512, (qc + 1) * 512)
                si = si0
                for ktg in ktgs:
                    eps = [
                        peps.tile([P, 1024], F32, tag="eps", name=f"eps{hp}")
                        for hp in range(2)
                    ]
                    for j in range(2):
                        kt = ktg * 2 + j
                        ks = slice(kt * P, (kt + 1) * P)
                        for hp in range(2):
                            rows = slice(hp * D, (hp + 1) * D)
                            nc.tensor.matmul(
                                eps[hp][:, j * 512 : (j + 1) * 512],
                                kT_sb[rows, pr, ks],
                                qT_sb[rows, pr, qs],
                                start=True,
                                stop=True,
                            )
                    sT = [
                        spool.tile([P, 1024], BF16, tag="sT", name=f"sT{hp}")
                        for hp in range(2)
                    ]
                    for hp in range(2):
                        nc.scalar.activation(sT[hp], eps[hp], EXP)
                    for j in range(2):
                        kt = ktg * 2 + j
                        for hp in range(2):
                            nc.tensor.matmul(
                                po[hp],
                                v_sb[:, kt, 2 * pr + hp, :],
                                sT[hp][:, j * 512 : (j + 1) * 512],
                                start=(kt == 0),
                                stop=(kt == KT - 1),
                            )
                    if slots is not None:
                        for u in slots[si]:
                            u()
                        si += 1

            def normalize(qc, pr, po):
                qs = slice(qc * 512, (qc + 1) * 512)
                for hp in range(2):
                    rinv = npool.tile([1, 512], BF16, tag="rinv")
                    nc.vector.reciprocal(rinv, po[hp][D : D + 1, :])
                    o_tmp = npool.tile([D, 512], F32, tag="otmp")
                    nc.vector.tensor_copy(o_tmp, po[hp][0:D, :])
                    bc = pbc.tile([D, 512], F32, tag="bc")
                    nc.tensor.matmul(
                        bc, ones1_b[:, 0:D], rinv, start=True, stop=True
                    )
                    nc.vector.tensor_mul(
                        oT_sb[hp * D : (hp + 1) * D, pr, qs], o_tmp, bc
                    )

            def new_po():
                return [
                    ppo.tile([D + 1, 512], F32, tag="po", name=f"po{hp}")
                    for hp in range(2)
                ]

            # ---- emission: interleave qc=0 attention into the k/v loads so
            # ScalarE starts exp'ing as soon as the first k/v tiles land ----
            emit_kv_chunk(0)
            for u in q_proj_units(0):
                u()
            emit_kv_chunk(1)
            po0 = new_po()
            attn_groups(0, 0, po0, range(0, 4))        # ktiles 0-7 (kv 0,1)
            emit_kv_chunk(2)
            attn_groups(0, 0, po0, range(4, 6))        # ktiles 8-11 (kv 2)
            emit_kv_chunk(3)
            attn_groups(0, 0, po0, range(6, 8))        # ktiles 12-15 (kv 3)
            normalize(0, 0, po0)
            # q-chunk-1 projection interleaved into qc0/pr1 attention
            d0 = q_proj_units(1)
            slots0 = [[] for _ in range(KT // 2)]
            for i, u in enumerate(d0):
                slots0[(i * (KT // 2)) // len(d0)].append(u)
            po1 = new_po()
            attn_groups(0, 1, po1, range(0, 8), slots0, 0)
            normalize(0, 1, po1)

            # ---- remaining q-chunks with deferred work interleaved ----
            for qc in range(1, NCH):
                deferred = q_proj_units(qc + 1) if qc + 1 < NCH else []
                deferred += outproj_units(qc - 1)
                nslots = DCH * (KT // 2)
                slots = [[] for _ in range(nslots)]
                for i, u in enumerate(deferred):
                    slots[(i * nslots) // max(len(deferred), 1)].append(u)
                for pr in range(DCH):
                    po = new_po()
                    attn_groups(qc, pr, po, range(KT // 2), slots,
                                pr * (KT // 2))
                    normalize(qc, pr, po)
            # tail: output projection of the last q-chunk
            for u in outproj_units(NCH - 1):
                u()

    nc.compile()
    return nc


def _shard_inputs(queries, keys, values, Wq, bq, Wk, bk, Wv, bv, Wp):
    """Host-side shard/layout prep: one flat fp16 blob per core holding
    feature-major activations, transposed weight shards, and biases."""
    f32 = np.float32
    f16 = np.float16
    # per-core x: feature-major slice of 512 positions (rank r = c % HG)
    xT = {}
    for name, x in (("xq", queries), ("xk", keys), ("xv", values)):
        xT[name] = [
            np.ascontiguousarray(np.asarray(x[b], f32).T.astype(f16))
            for b in range(B)
        ]
    Wq, Wk, Wv = (np.asarray(w, f32) for w in (Wq, Wk, Wv))
    Wp = np.asarray(Wp, f32)
    bq, bk, bv = (np.asarray(b_, f32) for b_ in (bq, bk, bv))
    maps = []
    for c in range(NCORES):
        b, hg = c // HG, c % HG
        rows = slice(hg * DH, (hg + 1) * DH)
        ps = slice((c % HG) * 512, (c % HG + 1) * 512)
        blob = np.concatenate([
            np.ascontiguousarray(xT["xq"][b][:, ps]).ravel(),
            np.ascontiguousarray(xT["xk"][b][:, ps]).ravel(),
            np.ascontiguousarray(xT["xv"][b][:, ps]).ravel(),
            np.ascontiguousarray(Wq[rows].T.astype(f16)).ravel(),
            np.ascontiguousarray(Wk[rows].T.astype(f16)).ravel(),
            np.ascontiguousarray(Wv[rows].T.astype(f16)).ravel(),
            np.ascontiguousarray(Wp[:, rows].T.astype(f16)).ravel(),
            bq[rows].astype(f16), bk[rows].astype(f16), bv[rows].astype(f16),
        ])
        assert blob.shape[0] == SZ_BLOB
        maps.append({"blob": blob})
    return maps


def kernel(queries, keys, values, Wq, bq, Wk, bk, Wv, bv, Wp, bp):
    from concourse.bass_utils import run_bass_kernel_spmd

    if "nc" not in _CACHE:
        _CACHE["nc"] = _build_program()
    nc = _CACHE["nc"]

    in_maps = _shard_inputs(queries, keys, values, Wq, bq, Wk, bk, Wv, bv, Wp)

    res = run_bass_kernel_spmd(nc, in_maps, list(range(NCORES)))

    out = np.zeros((B, N, E), np.float32)
    for c in range(NCORES):
        out[c // HG] += res.results[c]["out"].astype(np.float32)
    out += np.asarray(bp, np.float32)
    return out

